# revision 24
# baseline (speedup 1.0000x reference)
"""CharacterCNNEmbedding Trainium2 Bass kernel.

Full inputs -> full output [8, 512, 2048]. Data-parallel over 8 NeuronCores
(512 words each). Per core:
  - fp16 one-hot (DVE is_equal, 512-col tiles for 4x mode) + fp16
    gather-matmuls build char embeddings, ACT-copied straight into the
    merged conv moving matrix rows 0:16
  - 6 quarter-level shift DMAs replicate the embedding into rows 16..111
    (tap offsets 1..6); row 112 holds ones (folds conv bias), loaded once
  - one fp16 matmul per (filter-tile, position-chunk), K=113, evaluates all
    7 conv widths at 44 positions; 17 masked tail matmuls cover positions
    44..49 for widths < 7
  - relu+max-pool split between DVE direct reduce and ACT-relu->fp16 staging
    with DVE 2x fold tree
  - 2 highway layers + projection as 2048x2048 fp16 matmuls; weights
    streamed as one [128, 2048] DMA per (matrix, out-tile) (80 DMAs total
    instead of 1280 per-k-tile loads), ACT relu/sigmoid with fused bias,
    DVE gate combine
"""
import sys

sys.path.insert(0, "/opt/trn_rl_repo")
import numpy as np

B, S, L = 8, 512, 50
LP = 56                      # padded word length (shifts stay in-word)
NCORES = 8
W = 512                      # words per core
QW = 128                     # words per quarter
NQ = W // QW
QCOLS = QW * LP              # 7168
COLS = W * LP                # 28672
CD = 16
TOTAL_F = 2048
NMT = TOTAL_F // 128         # 16 filter tiles
VCHUNKS = 3                  # vocab 262 -> 3 chunks of 128
ET = 512                     # emb tile cols (even -> DVE 4x is_equal)
NE = QCOLS // ET             # 14
CHW = 8                      # conv chunk words (352 cols)
NCH = QW // CHW              # 16 chunks per quarter
GRP = 2                      # chunks per PSUM group
FILTERS = [(1, 32), (2, 32), (3, 64), (4, 128), (5, 256), (6, 512), (7, 1024)]
TAIL_TILES = (
    [(44, j) for j in range(8)]
    + [(45, j) for j in range(4)]
    + [(46, j) for j in range(2)]
    + [(47, 0), (48, 0), (49, 0)]
)
DIRECT_M = (5, 11)            # m-tiles on DVE direct-reduce path; rest go
                              # ACT-relu staging + DVE fold tree (interleaved
                              # so both consumer engines run concurrently)

_prog_cache = {}


def _build_program(n_words=W):
    import concourse.tile as tile
    from concourse import bacc, mybir

    F32 = mybir.dt.float32
    F16 = mybir.dt.float16
    F8 = mybir.dt.float8e4
    DR = mybir.MatmulPerfMode.DoubleRow
    nq = n_words // QW

    nc = bacc.Bacc("TRN2", target_bir_lowering=False, debug=False)

    idsb_d = nc.dram_tensor("idsb", [nq, 128, QCOLS], F16, kind="ExternalInput").ap()
    iota_d = nc.dram_tensor("iota", [128, VCHUNKS], F32, kind="ExternalInput").ap()
    tab_d = nc.dram_tensor("tab", [128, VCHUNKS * CD], F16, kind="ExternalInput").ap()
    ones_d = nc.dram_tensor("ones", [1, QCOLS], F16, kind="ExternalInput").ap()
    convw_d = nc.dram_tensor("convw", [113, TOTAL_F], F16, kind="ExternalInput").ap()
    tailw_d = nc.dram_tensor(
        "tailw", [113, len(TAIL_TILES) * 128], F16, kind="ExternalInput"
    ).ap()
    wstack_d = nc.dram_tensor(
        "wstack", [5, NMT, 128, NMT * 128], F16, kind="ExternalInput"
    ).ap()
    wg8_d = nc.dram_tensor(
        "wg8", [2, NMT, 128, NMT * 128], F8, kind="ExternalInput"
    ).ap()
    hbT_d = nc.dram_tensor("hbT", [128, 5 * NMT], F32, kind="ExternalInput").ap()
    out_d = nc.dram_tensor("out", [TOTAL_F, n_words], F32, kind="ExternalOutput").ap()

    AF = mybir.ActivationFunctionType
    OP = mybir.AluOpType
    AX = mybir.AxisListType

    with tile.TileContext(nc) as tc:
        with (
            tc.tile_pool(name="const", bufs=1) as cpool,
            tc.tile_pool(name="h", bufs=1) as hp,
            tc.tile_pool(name="ws", bufs=4) as wsp,
            tc.tile_pool(name="tmp", bufs=2) as tmpp,
            tc.tile_pool(name="outp", bufs=2) as outp,
        ):
            # emb-path consts first so the first quarter starts ASAP
            iota_sb = cpool.tile([128, VCHUNKS], F32)
            nc.sync.dma_start(iota_sb[:], iota_d[:])
            tab_sb = cpool.tile([128, VCHUNKS * CD], F16)
            nc.sync.dma_start(tab_sb[:], tab_d[:])

            # two persistent mov buffers (ones row loaded once each)
            movs = [cpool.tile([128, QCOLS], F16, name=f"mov{i}") for i in range(2)]
            for mv in movs:
                nc.sync.dma_start(mv[112:113, :], ones_d[:])

            convw_sb = cpool.tile([113, TOTAL_F], F16)
            nc.scalar.dma_start(convw_sb[:], convw_d[:])
            tailw_sb = cpool.tile([113, len(TAIL_TILES) * 128], F16)
            nc.scalar.dma_start(tailw_sb[:], tailw_d[:])
            hbT_sb = cpool.tile([128, 5 * NMT], F32)
            nc.scalar.dma_start(hbT_sb[:], hbT_d[:])

            h0 = [hp.tile([128, W], F16, name=f"ha_{k}") for k in range(NMT)]

            # ---------------- conv + embedding phase ----------------
            with (
                tc.tile_pool(name="ids", bufs=2) as idsp,
                tc.tile_pool(name="oh", bufs=9) as ohp,
                tc.tile_pool(name="stg", bufs=2) as stgp,
                tc.tile_pool(name="fold", bufs=3) as foldp,
                tc.tile_pool(name="embp", bufs=2, space="PSUM") as embpp,
                tc.tile_pool(name="convp", bufs=3, space="PSUM") as convpp,
            ):
                def get_ids(q):
                    if q not in ids_tiles:
                        ids_tiles[q] = idsp.tile([128, QCOLS], F16, name="ids")
                    return ids_tiles[q]

                def emit_ids_chunk(q, ic):
                    c_lo = ic * (QCOLS // 4)
                    c_hi = (ic + 1) * (QCOLS // 4)
                    nc.sync.dma_start(
                        get_ids(q)[:, c_lo:c_hi], idsb_d[q][:, c_lo:c_hi]
                    )

                def emit_etile(q, e):
                    c0 = ET * e
                    embp = embpp.tile([16, ET], F32, name="embp")
                    for v in range(VCHUNKS):
                        oh = ohp.tile([128, ET], F16, name="oh")
                        nc.vector.tensor_scalar(
                            oh[:],
                            get_ids(q)[:, c0 : c0 + ET],
                            iota_sb[:, v : v + 1],
                            None,
                            op0=OP.is_equal,
                        )
                        nc.tensor.matmul(
                            embp[:],
                            tab_sb[:, v * CD : (v + 1) * CD],
                            oh[:],
                            start=(v == 0),
                            stop=(v == VCHUNKS - 1),
                        )
                    nc.scalar.activation(
                        movs[q % 2][0:16, c0 : c0 + ET], embp[:], AF.Copy
                    )

                def emit_shifts(q):
                    # rows 16dt..16dt+16 = emb shifted left dt chars
                    mov = movs[q % 2]
                    for dt in range(1, 7):
                        nc.sync.dma_start(
                            mov[16 * dt : 16 * dt + 16, 0 : QCOLS - dt],
                            mov[0:16, dt:QCOLS],
                        )

                tails_of = {}
                for idx, (tt, j) in enumerate(TAIL_TILES):
                    tails_of.setdefault(j, []).append((idx, tt))

                ids_tiles = {}
                for q in range(nq):
                    if q == 0:
                        for ic in range(4):
                            emit_ids_chunk(0, ic)
                        for e in range(NE):
                            emit_etile(0, e)
                        emit_shifts(0)

                    mov = movs[q % 2]
                    movv = mov[0:113, :].rearrange("p (w l) -> p w l", l=LP)

                    # conv per filter tile, with next quarter's embedding
                    # e-tiles and this quarter's tail matmuls interleaved to
                    # avoid burst serialization at quarter boundaries
                    for m in range(NMT):
                        staged = m not in DIRECT_M
                        wsl = convw_sb[:, m * 128 : (m + 1) * 128]
                        if staged:
                            stg = stgp.tile([128, QW, 44], F16, name="stg")
                        else:
                            rm = foldp.tile([128, QW], F32, name="rm")
                        ci = 0
                        while ci < NCH:
                            ng = min(GRP, NCH - ci)
                            cp = convpp.tile([128, GRP, 512], F32, name="cv")
                            for i in range(ng):
                                nc.tensor.matmul(
                                    cp[:, i, 0 : CHW * 44],
                                    wsl,
                                    movv[:, (ci + i) * CHW : (ci + i + 1) * CHW, 0:44],
                                    start=True,
                                    stop=True,
                                )
                            src4 = cp[:, 0:ng, 0 : CHW * 44].rearrange(
                                "p c (w l) -> p c w l", l=44
                            )
                            wlo = ci * CHW
                            whi = (ci + ng) * CHW
                            if staged:
                                nc.scalar.activation(
                                    stg[:, wlo:whi, :].rearrange(
                                        "p (c w) l -> p c w l", c=ng
                                    ),
                                    src4,
                                    AF.Relu,
                                )
                            else:
                                nc.vector.tensor_reduce(
                                    rm[:, wlo:whi].rearrange("p (c w) -> p c w", c=ng),
                                    src4,
                                    op=OP.max,
                                    axis=AX.X,
                                )
                            ci += ng
                        hslice = h0[m][:, q * QW : (q + 1) * QW]
                        if staged:
                            # 44 -> 24 via overlapped max (cols 20..23 counted
                            # twice; harmless for max), then 24 -> 12 -> 6 -> 1
                            t24 = foldp.tile([128, QW, 24], F16, name="t24")
                            nc.vector.tensor_max(
                                t24[:], stg[:, :, 0:24], stg[:, :, 20:44]
                            )
                            f12 = foldp.tile([128, QW, 12], F16, name="fold")
                            nc.vector.tensor_max(
                                f12[:], t24[:, :, 0:12], t24[:, :, 12:24]
                            )
                            f6 = foldp.tile([128, QW, 6], F16, name="fold")
                            nc.vector.tensor_max(f6[:], f12[:, :, 0:6], f12[:, :, 6:12])
                            nc.vector.tensor_reduce(
                                hslice, f6, op=OP.max, axis=AX.X
                            )
                        else:
                            nc.scalar.activation(hslice, rm[:], AF.Relu)

                        # tails for this m-tile (masked weights, pos 44..49)
                        for idx, tt in tails_of.get(m, []):
                            tp = convpp.tile([128, GRP, 512], F32, name="cv")
                            nc.tensor.matmul(
                                tp[:, 0, 0:QW],
                                tailw_sb[:, idx * 128 : (idx + 1) * 128],
                                movv[:, :, tt],
                                start=True,
                                stop=True,
                            )
                            nc.vector.tensor_max(hslice, hslice, tp[:, 0, 0:QW])

                        # interleave next quarter's embedding work
                        if q + 1 < nq:
                            if m < 4:
                                emit_ids_chunk(q + 1, m)
                            if m < NE:
                                emit_etile(q + 1, m)
                            if m == NE:
                                emit_shifts(q + 1)

            # ---------------- highway + projection ----------------
            with tc.tile_pool(name="hw", bufs=4, space="PSUM") as hwp:
                hin = h0
                h8 = [hp.tile([128, 2, W], F8, name=f"h8_{t}") for t in range(NMT // 2)]
                for layer in range(2):
                    # fp8 pair-interleaved copy of hin for DoubleRow gate chain
                    for k in range(NMT):
                        nc.vector.tensor_copy(h8[k // 2][:, k % 2, :], hin[k][:])
                    hout = [
                        hp.tile([128, W], F16, name=f"h{'b' if layer == 0 else 'a'}_{k}")
                        for k in range(NMT)
                    ]
                    for m in range(NMT):
                        wt = wsp.tile([128, NMT * 128], F16, name="wt")
                        nc.gpsimd.dma_start(wt[:], wstack_d[2 * layer, m])
                        wg = wsp.tile([128, NMT * 128], F8, name="wg")
                        nc.gpsimd.dma_start(wg[:], wg8_d[layer, m])
                        wgv = wg[:].rearrange("p (t s c) -> p t s c", s=2, c=128)
                        pt = hwp.tile([128, 512], F32, name="pt")
                        pg = hwp.tile([128, 512], F32, name="pg")
                        for k in range(NMT):
                            nc.tensor.matmul(
                                pt[:, 0:n_words],
                                wt[:, k * 128 : (k + 1) * 128],
                                hin[k][:],
                                start=(k == 0),
                                stop=(k == NMT - 1),
                            )
                        for t in range(NMT // 2):
                            nc.tensor.matmul(
                                pg[:, 0:n_words],
                                wgv[:, t],
                                h8[t][:],
                                start=(t == 0),
                                stop=(t == NMT // 2 - 1),
                                perf_mode=DR,
                            )
                        t_sb = tmpp.tile([128, W], F16, name="t_sb")
                        nc.scalar.activation(
                            t_sb[:],
                            pt[:, 0:n_words],
                            AF.Relu,
                            bias=hbT_sb[:, 2 * layer * NMT + m : 2 * layer * NMT + m + 1],
                        )
                        g_sb = tmpp.tile([128, W], F16, name="g_sb")
                        nc.scalar.activation(
                            g_sb[:],
                            pg[:, 0:n_words],
                            AF.Sigmoid,
                            bias=hbT_sb[
                                :, (2 * layer + 1) * NMT + m : (2 * layer + 1) * NMT + m + 1
                            ],
                        )
                        d_sb = tmpp.tile([128, W], F16, name="de")
                        nc.vector.tensor_sub(d_sb[:], t_sb[:], hin[m][:])
                        e_sb = tmpp.tile([128, W], F16, name="de")
                        nc.vector.tensor_mul(e_sb[:], g_sb[:], d_sb[:])
                        nc.vector.tensor_add(hout[m][:], hin[m][:], e_sb[:])
                    hin = hout

                for m in range(NMT):
                    wp = wsp.tile([128, NMT * 128], F16, name="wt")
                    nc.gpsimd.dma_start(wp[:], wstack_d[4, m])
                    pp = hwp.tile([128, 512], F32, name="pt")
                    for k in range(NMT):
                        nc.tensor.matmul(
                            pp[:, 0:n_words],
                            wp[:, k * 128 : (k + 1) * 128],
                            hin[k][:],
                            start=(k == 0),
                            stop=(k == NMT - 1),
                        )
                    o_sb = outp.tile([128, W], F32, name="o_sb")
                    nc.vector.tensor_scalar_add(
                        o_sb[:], pp[:, 0:n_words],
                        hbT_sb[:, 4 * NMT + m : 4 * NMT + m + 1],
                    )
                    nc.sync.dma_start(out_d[m * 128 : (m + 1) * 128, :], o_sb[:])

    nc.compile()
    return nc


def _prep_weights(inputs):
    """Host-side weight marshalling (layout + fp16 rounding)."""
    f32 = np.float32
    table = np.asarray(inputs["char_table"], f32).copy()
    table[0] = 0.0
    tab16 = np.zeros((128, VCHUNKS * CD), np.float16)
    for v in range(VCHUNKS):
        rows = table[128 * v : min(128 * (v + 1), table.shape[0])]
        tab16[: rows.shape[0], v * CD : (v + 1) * CD] = rows.astype(np.float16)
    iota = np.zeros((128, VCHUNKS), f32)
    for v in range(VCHUNKS):
        iota[:, v] = np.arange(128) + 128 * v

    convw = np.zeros((113, TOTAL_F), f32)
    offs = np.concatenate([[0], np.cumsum([nf for _, nf in FILTERS])])
    widths = np.repeat([w for w, _ in FILTERS], [nf for _, nf in FILTERS])
    for i, (w, nf) in enumerate(FILTERS):
        cw = np.asarray(inputs[f"conv_w{i}"], f32)  # [nf, 16, w]
        for dt in range(w):
            convw[dt * CD : (dt + 1) * CD, offs[i] : offs[i] + nf] = cw[:, :, dt].T
        convw[112, offs[i] : offs[i] + nf] = np.asarray(inputs[f"conv_b{i}"], f32)
    tailw = np.zeros((113, len(TAIL_TILES) * 128), f32)
    for idx, (tt, j) in enumerate(TAIL_TILES):
        blk = convw[:, 128 * j : 128 * (j + 1)].copy()
        blk[:, widths[128 * j : 128 * (j + 1)] > (50 - tt)] = 0.0
        tailw[:, 128 * idx : 128 * (idx + 1)] = blk

    wstack = np.stack(
        [
            np.asarray(inputs["hw0_tw"], f32).T,
            np.asarray(inputs["hw0_gw"], f32).T,
            np.asarray(inputs["hw1_tw"], f32).T,
            np.asarray(inputs["hw1_gw"], f32).T,
            np.asarray(inputs["proj_w"], f32).T,
        ]
    ).astype(np.float16)
    # pre-tile: [5, 2048, 2048] -> [5, mt, 128(kin rows), kt*128(mout cols)]
    # so one [128, 2048] DMA fetches all 16 k-tiles for out-tile mt
    wstack_t = np.ascontiguousarray(
        wstack.reshape(5, NMT, 128, NMT, 128)
        .transpose(0, 3, 2, 1, 4)
        .reshape(5, NMT, 128, NMT * 128)
    )
    hb = [
        np.asarray(inputs["hw0_tb"], f32),
        np.asarray(inputs["hw0_gb"], f32),
        np.asarray(inputs["hw1_tb"], f32),
        np.asarray(inputs["hw1_gb"], f32),
        np.asarray(inputs["proj_b"], f32),
    ]
    hbT = np.zeros((128, 5 * NMT), f32)
    for p_i in range(5):
        for m in range(NMT):
            hbT[:, p_i * NMT + m] = hb[p_i][m * 128 : (m + 1) * 128]

    import ml_dtypes

    wg8 = np.zeros((2, NMT, 128, NMT * 128), ml_dtypes.float8_e4m3)
    for li, gname in enumerate(["hw0_gw", "hw1_gw"]):
        GT = np.asarray(inputs[gname], f32).T  # [in, out]
        # [t, s, r, m, c] -> [m, r, t, s, c]
        pk = (
            GT.reshape(NMT // 2, 2, 128, NMT, 128)
            .transpose(3, 2, 0, 1, 4)
            .reshape(NMT, 128, NMT * 128)
        )
        wg8[li] = np.clip(pk, -240, 240).astype(ml_dtypes.float8_e4m3)

    return {
        "iota": iota,
        "tab": tab16,
        "ones": np.ones((1, QCOLS), np.float16),
        "convw": convw.astype(np.float16),
        "tailw": tailw.astype(np.float16),
        "wstack": wstack_t,
        "wg8": wg8,
        "hbT": hbT,
    }


def _prep_ids(char_ids):
    ids = np.asarray(char_ids).reshape(B * S, L)
    ids_pad = np.zeros((B * S, LP), np.int32)
    ids_pad[:, :L] = ids
    nq = W // QW
    per_core = []
    for c in range(NCORES):
        flat = ids_pad[c * W : (c + 1) * W].reshape(-1).astype(np.float16)
        chunks = np.zeros((nq, 128, QCOLS), np.float16)
        for q in range(nq):
            chunks[q, :, :] = flat[q * QCOLS : (q + 1) * QCOLS][None, :]
        per_core.append(chunks)
    return per_core


def _run(inputs, trace=False):
    from concourse.bass_utils import run_bass_kernel_spmd

    if "prog" not in _prog_cache:
        _prog_cache["prog"] = _build_program()
    nc = _prog_cache["prog"]

    shared = _prep_weights(inputs)
    idsb = _prep_ids(inputs["char_ids"])
    in_maps = [dict(shared, idsb=idsb[c]) for c in range(NCORES)]
    br = run_bass_kernel_spmd(nc, in_maps, list(range(NCORES)), trace=trace)
    outs = [br.results[c]["out"] for c in range(NCORES)]  # [2048, 512] each
    full = np.concatenate([o.T for o in outs], axis=0)  # [4096, 2048]
    return full.reshape(B, S, TOTAL_F).astype(np.float32), br


def kernel(**inputs):
    out, _ = _run(inputs, trace=False)
    return out


# revision 25
# speedup vs baseline: 1.0621x; 1.0621x over previous
"""CharacterCNNEmbedding Trainium2 Bass kernel.

Full inputs -> full output [8, 512, 2048]. Data-parallel over 8 NeuronCores
(512 words each). Per core:
  - fp16 one-hot (DVE is_equal, 512-col tiles for 4x mode) + fp16
    gather-matmuls build char embeddings, ACT-copied straight into the
    merged conv moving matrix rows 0:16
  - 6 quarter-level shift DMAs replicate the embedding into rows 16..111
    (tap offsets 1..6); row 112 holds ones (folds conv bias), loaded once
  - one fp16 matmul per (filter-tile, position-chunk), K=113, evaluates all
    7 conv widths at 44 positions; 17 masked tail matmuls cover positions
    44..49 for widths < 7
  - relu+max-pool split between DVE direct reduce and ACT-relu->fp16 staging
    with DVE 2x fold tree
  - 2 highway layers + projection as 2048x2048 fp16 matmuls; weights
    streamed as one [128, 2048] DMA per (matrix, out-tile) (80 DMAs total
    instead of 1280 per-k-tile loads), ACT relu/sigmoid with fused bias,
    DVE gate combine
"""
import sys

sys.path.insert(0, "/opt/trn_rl_repo")
import numpy as np

B, S, L = 8, 512, 50
LP = 56                      # padded word length (shifts stay in-word)
NCORES = 8
W = 512                      # words per core
QW = 128                     # words per quarter
NQ = W // QW
QCOLS = QW * LP              # 7168
COLS = W * LP                # 28672
CD = 16
TOTAL_F = 2048
NMT = TOTAL_F // 128         # 16 filter tiles
VCHUNKS = 3                  # vocab 262 -> 3 chunks of 128
ET = 512                     # emb tile cols (even -> DVE 4x is_equal)
NE = QCOLS // ET             # 14
CHW = 8                      # conv chunk words (352 cols)
NCH = QW // CHW              # 16 chunks per quarter
GRP = 2                      # chunks per PSUM group
FILTERS = [(1, 32), (2, 32), (3, 64), (4, 128), (5, 256), (6, 512), (7, 1024)]
TAIL_TILES = (
    [(44, j) for j in range(8)]
    + [(45, j) for j in range(4)]
    + [(46, j) for j in range(2)]
    + [(47, 0), (48, 0), (49, 0)]
)
DIRECT_M = (5, 11)            # m-tiles on DVE direct-reduce path; rest go
                              # ACT-relu staging + DVE fold tree (interleaved
                              # so both consumer engines run concurrently)

_prog_cache = {}


def _build_program(n_words=W):
    import concourse.tile as tile
    from concourse import bacc, mybir

    F32 = mybir.dt.float32
    F16 = mybir.dt.float16
    F8 = mybir.dt.float8e4
    DR = mybir.MatmulPerfMode.DoubleRow
    nq = n_words // QW

    nc = bacc.Bacc("TRN2", target_bir_lowering=False, debug=False)

    idsb_d = nc.dram_tensor("idsb", [nq, 128, QCOLS], F16, kind="ExternalInput").ap()
    iota_d = nc.dram_tensor("iota", [128, VCHUNKS], F32, kind="ExternalInput").ap()
    tab_d = nc.dram_tensor("tab", [128, VCHUNKS * CD], F16, kind="ExternalInput").ap()
    ones_d = nc.dram_tensor("ones", [1, QCOLS], F16, kind="ExternalInput").ap()
    convw_d = nc.dram_tensor("convw", [113, TOTAL_F], F16, kind="ExternalInput").ap()
    tailw_d = nc.dram_tensor(
        "tailw", [113, len(TAIL_TILES) * 128], F16, kind="ExternalInput"
    ).ap()
    wstack_d = nc.dram_tensor(
        "wstack", [5, NMT, 128, NMT * 128], F16, kind="ExternalInput"
    ).ap()
    wg8_d = nc.dram_tensor(
        "wg8", [2, NMT, 128, NMT * 128], F8, kind="ExternalInput"
    ).ap()
    hbT_d = nc.dram_tensor("hbT", [128, 5 * NMT], F32, kind="ExternalInput").ap()
    out_d = nc.dram_tensor("out", [TOTAL_F, n_words], F32, kind="ExternalOutput").ap()

    AF = mybir.ActivationFunctionType
    OP = mybir.AluOpType
    AX = mybir.AxisListType

    with tile.TileContext(nc) as tc:
        with (
            tc.tile_pool(name="const", bufs=1) as cpool,
            tc.tile_pool(name="h", bufs=1) as hp,
            tc.tile_pool(name="ws", bufs=4) as wsp,
            tc.tile_pool(name="tmp", bufs=2) as tmpp,
            tc.tile_pool(name="outp", bufs=2) as outp,
        ):
            # emb-path consts first so the first quarter starts ASAP
            iota_sb = cpool.tile([128, VCHUNKS], F32)
            nc.sync.dma_start(iota_sb[:], iota_d[:])
            tab_sb = cpool.tile([128, VCHUNKS * CD], F16)
            nc.sync.dma_start(tab_sb[:], tab_d[:])

            # two persistent mov buffers (ones row loaded once each)
            movs = [cpool.tile([128, QCOLS], F16, name=f"mov{i}") for i in range(2)]
            for mv in movs:
                nc.sync.dma_start(mv[112:113, :], ones_d[:])

            convw_sb = cpool.tile([113, TOTAL_F], F16)
            nc.scalar.dma_start(convw_sb[:], convw_d[:])
            tailw_sb = cpool.tile([113, len(TAIL_TILES) * 128], F16)
            nc.scalar.dma_start(tailw_sb[:], tailw_d[:])
            hbT_sb = cpool.tile([128, 5 * NMT], F32)
            nc.scalar.dma_start(hbT_sb[:], hbT_d[:])

            h0 = [hp.tile([128, W], F16, name=f"ha_{k}") for k in range(NMT)]

            # ---------------- conv + embedding phase ----------------
            with (
                tc.tile_pool(name="ids", bufs=2) as idsp,
                tc.tile_pool(name="oh", bufs=9) as ohp,
                tc.tile_pool(name="stg", bufs=2) as stgp,
                tc.tile_pool(name="fold", bufs=3) as foldp,
                tc.tile_pool(name="embp", bufs=2, space="PSUM") as embpp,
                tc.tile_pool(name="convp", bufs=3, space="PSUM") as convpp,
            ):
                for q in range(nq):
                    mov = movs[q % 2]
                    ids_sb = idsp.tile([128, QCOLS], F16, name="ids")
                    for ic in range(4):
                        c_lo = ic * (QCOLS // 4)
                        c_hi = (ic + 1) * (QCOLS // 4)
                        nc.sync.dma_start(
                            ids_sb[:, c_lo:c_hi], idsb_d[q][:, c_lo:c_hi]
                        )

                    for e in range(NE):
                        c0 = ET * e
                        embp = embpp.tile([16, ET], F32, name="embp")
                        for v in range(VCHUNKS):
                            oh = ohp.tile([128, ET], F16, name="oh")
                            nc.vector.tensor_scalar(
                                oh[:],
                                ids_sb[:, c0 : c0 + ET],
                                iota_sb[:, v : v + 1],
                                None,
                                op0=OP.is_equal,
                            )
                            nc.tensor.matmul(
                                embp[:],
                                tab_sb[:, v * CD : (v + 1) * CD],
                                oh[:],
                                start=(v == 0),
                                stop=(v == VCHUNKS - 1),
                            )
                        nc.scalar.activation(
                            mov[0:16, c0 : c0 + ET], embp[:], AF.Copy
                        )

                    # quarter-level shift DMAs: rows 16dt..16dt+16 = emb
                    # shifted left dt chars (taps 1..6; tap 0 is rows 0:16)
                    for dt in range(1, 7):
                        nc.sync.dma_start(
                            mov[16 * dt : 16 * dt + 16, 0 : QCOLS - dt],
                            mov[0:16, dt:QCOLS],
                        )

                    movv = mov[0:113, :].rearrange("p (w l) -> p w l", l=LP)

                    # conv per filter tile
                    for m in range(NMT):
                        staged = m not in DIRECT_M
                        wsl = convw_sb[:, m * 128 : (m + 1) * 128]
                        if staged:
                            stg = stgp.tile([128, QW, 44], F16, name="stg")
                        else:
                            rm = foldp.tile([128, QW], F32, name="rm")
                        ci = 0
                        while ci < NCH:
                            ng = min(GRP, NCH - ci)
                            cp = convpp.tile([128, GRP, 512], F32, name="cv")
                            for i in range(ng):
                                nc.tensor.matmul(
                                    cp[:, i, 0 : CHW * 44],
                                    wsl,
                                    movv[:, (ci + i) * CHW : (ci + i + 1) * CHW, 0:44],
                                    start=True,
                                    stop=True,
                                )
                            src4 = cp[:, 0:ng, 0 : CHW * 44].rearrange(
                                "p c (w l) -> p c w l", l=44
                            )
                            wlo = ci * CHW
                            whi = (ci + ng) * CHW
                            if staged:
                                nc.scalar.activation(
                                    stg[:, wlo:whi, :].rearrange(
                                        "p (c w) l -> p c w l", c=ng
                                    ),
                                    src4,
                                    AF.Relu,
                                )
                            else:
                                nc.vector.tensor_reduce(
                                    rm[:, wlo:whi].rearrange("p (c w) -> p c w", c=ng),
                                    src4,
                                    op=OP.max,
                                    axis=AX.X,
                                )
                            ci += ng
                        hslice = h0[m][:, q * QW : (q + 1) * QW]
                        if staged:
                            # 44 -> 24 via overlapped max (cols 20..23 counted
                            # twice; harmless for max), then 24 -> 12 -> 6 -> 1
                            t24 = foldp.tile([128, QW, 24], F16, name="t24")
                            nc.vector.tensor_max(
                                t24[:], stg[:, :, 0:24], stg[:, :, 20:44]
                            )
                            f12 = foldp.tile([128, QW, 12], F16, name="fold")
                            nc.vector.tensor_max(
                                f12[:], t24[:, :, 0:12], t24[:, :, 12:24]
                            )
                            f6 = foldp.tile([128, QW, 6], F16, name="fold")
                            nc.vector.tensor_max(f6[:], f12[:, :, 0:6], f12[:, :, 6:12])
                            nc.vector.tensor_reduce(
                                hslice, f6, op=OP.max, axis=AX.X
                            )
                        else:
                            nc.scalar.activation(hslice, rm[:], AF.Relu)

                    # tail positions 44..49 (masked weights)
                    for idx, (tt, j) in enumerate(TAIL_TILES):
                        tp = convpp.tile([128, GRP, 512], F32, name="cv")
                        nc.tensor.matmul(
                            tp[:, 0, 0:QW],
                            tailw_sb[:, idx * 128 : (idx + 1) * 128],
                            movv[:, :, tt],
                            start=True,
                            stop=True,
                        )
                        hs = h0[j][:, q * QW : (q + 1) * QW]
                        nc.vector.tensor_max(hs, hs, tp[:, 0, 0:QW])

            # ---------------- highway + projection ----------------
            with tc.tile_pool(name="hw", bufs=4, space="PSUM") as hwp:
                hin = h0
                h8 = [hp.tile([128, 2, W], F8, name=f"h8_{t}") for t in range(NMT // 2)]
                for layer in range(2):
                    # fp8 pair-interleaved copy of hin for DoubleRow gate chain
                    for k in range(NMT):
                        nc.vector.tensor_copy(h8[k // 2][:, k % 2, :], hin[k][:])
                    hout = [
                        hp.tile([128, W], F16, name=f"h{'b' if layer == 0 else 'a'}_{k}")
                        for k in range(NMT)
                    ]
                    for m in range(NMT):
                        wt = wsp.tile([128, NMT * 128], F16, name="wt")
                        nc.gpsimd.dma_start(wt[:], wstack_d[2 * layer, m])
                        wg = wsp.tile([128, NMT * 128], F8, name="wg")
                        nc.gpsimd.dma_start(wg[:], wg8_d[layer, m])
                        wgv = wg[:].rearrange("p (t s c) -> p t s c", s=2, c=128)
                        pt = hwp.tile([128, 512], F32, name="pt")
                        pg = hwp.tile([128, 512], F32, name="pg")
                        for k in range(NMT):
                            nc.tensor.matmul(
                                pt[:, 0:n_words],
                                wt[:, k * 128 : (k + 1) * 128],
                                hin[k][:],
                                start=(k == 0),
                                stop=(k == NMT - 1),
                            )
                        for t in range(NMT // 2):
                            nc.tensor.matmul(
                                pg[:, 0:n_words],
                                wgv[:, t],
                                h8[t][:],
                                start=(t == 0),
                                stop=(t == NMT // 2 - 1),
                                perf_mode=DR,
                            )
                        t_sb = tmpp.tile([128, W], F16, name="t_sb")
                        nc.scalar.activation(
                            t_sb[:],
                            pt[:, 0:n_words],
                            AF.Relu,
                            bias=hbT_sb[:, 2 * layer * NMT + m : 2 * layer * NMT + m + 1],
                        )
                        g_sb = tmpp.tile([128, W], F16, name="g_sb")
                        nc.scalar.activation(
                            g_sb[:],
                            pg[:, 0:n_words],
                            AF.Sigmoid,
                            bias=hbT_sb[
                                :, (2 * layer + 1) * NMT + m : (2 * layer + 1) * NMT + m + 1
                            ],
                        )
                        d_sb = tmpp.tile([128, W], F16, name="de")
                        nc.vector.tensor_sub(d_sb[:], t_sb[:], hin[m][:])
                        e_sb = tmpp.tile([128, W], F16, name="de")
                        nc.vector.tensor_mul(e_sb[:], g_sb[:], d_sb[:])
                        nc.vector.tensor_add(hout[m][:], hin[m][:], e_sb[:])
                    hin = hout

                for m in range(NMT):
                    wp = wsp.tile([128, NMT * 128], F16, name="wt")
                    nc.gpsimd.dma_start(wp[:], wstack_d[4, m])
                    pp = hwp.tile([128, 512], F32, name="pt")
                    for k in range(NMT):
                        nc.tensor.matmul(
                            pp[:, 0:n_words],
                            wp[:, k * 128 : (k + 1) * 128],
                            hin[k][:],
                            start=(k == 0),
                            stop=(k == NMT - 1),
                        )
                    o_sb = outp.tile([128, W], F32, name="o_sb")
                    nc.vector.tensor_scalar_add(
                        o_sb[:], pp[:, 0:n_words],
                        hbT_sb[:, 4 * NMT + m : 4 * NMT + m + 1],
                    )
                    nc.sync.dma_start(out_d[m * 128 : (m + 1) * 128, :], o_sb[:])

    nc.compile()
    return nc


def _prep_weights(inputs):
    """Host-side weight marshalling (layout + fp16 rounding)."""
    f32 = np.float32
    table = np.asarray(inputs["char_table"], f32).copy()
    table[0] = 0.0
    tab16 = np.zeros((128, VCHUNKS * CD), np.float16)
    for v in range(VCHUNKS):
        rows = table[128 * v : min(128 * (v + 1), table.shape[0])]
        tab16[: rows.shape[0], v * CD : (v + 1) * CD] = rows.astype(np.float16)
    iota = np.zeros((128, VCHUNKS), f32)
    for v in range(VCHUNKS):
        iota[:, v] = np.arange(128) + 128 * v

    convw = np.zeros((113, TOTAL_F), f32)
    offs = np.concatenate([[0], np.cumsum([nf for _, nf in FILTERS])])
    widths = np.repeat([w for w, _ in FILTERS], [nf for _, nf in FILTERS])
    for i, (w, nf) in enumerate(FILTERS):
        cw = np.asarray(inputs[f"conv_w{i}"], f32)  # [nf, 16, w]
        for dt in range(w):
            convw[dt * CD : (dt + 1) * CD, offs[i] : offs[i] + nf] = cw[:, :, dt].T
        convw[112, offs[i] : offs[i] + nf] = np.asarray(inputs[f"conv_b{i}"], f32)
    tailw = np.zeros((113, len(TAIL_TILES) * 128), f32)
    for idx, (tt, j) in enumerate(TAIL_TILES):
        blk = convw[:, 128 * j : 128 * (j + 1)].copy()
        blk[:, widths[128 * j : 128 * (j + 1)] > (50 - tt)] = 0.0
        tailw[:, 128 * idx : 128 * (idx + 1)] = blk

    wstack = np.stack(
        [
            np.asarray(inputs["hw0_tw"], f32).T,
            np.asarray(inputs["hw0_gw"], f32).T,
            np.asarray(inputs["hw1_tw"], f32).T,
            np.asarray(inputs["hw1_gw"], f32).T,
            np.asarray(inputs["proj_w"], f32).T,
        ]
    ).astype(np.float16)
    # pre-tile: [5, 2048, 2048] -> [5, mt, 128(kin rows), kt*128(mout cols)]
    # so one [128, 2048] DMA fetches all 16 k-tiles for out-tile mt
    wstack_t = np.ascontiguousarray(
        wstack.reshape(5, NMT, 128, NMT, 128)
        .transpose(0, 3, 2, 1, 4)
        .reshape(5, NMT, 128, NMT * 128)
    )
    hb = [
        np.asarray(inputs["hw0_tb"], f32),
        np.asarray(inputs["hw0_gb"], f32),
        np.asarray(inputs["hw1_tb"], f32),
        np.asarray(inputs["hw1_gb"], f32),
        np.asarray(inputs["proj_b"], f32),
    ]
    hbT = np.zeros((128, 5 * NMT), f32)
    for p_i in range(5):
        for m in range(NMT):
            hbT[:, p_i * NMT + m] = hb[p_i][m * 128 : (m + 1) * 128]

    import ml_dtypes

    wg8 = np.zeros((2, NMT, 128, NMT * 128), ml_dtypes.float8_e4m3)
    for li, gname in enumerate(["hw0_gw", "hw1_gw"]):
        GT = np.asarray(inputs[gname], f32).T  # [in, out]
        # [t, s, r, m, c] -> [m, r, t, s, c]
        pk = (
            GT.reshape(NMT // 2, 2, 128, NMT, 128)
            .transpose(3, 2, 0, 1, 4)
            .reshape(NMT, 128, NMT * 128)
        )
        wg8[li] = np.clip(pk, -240, 240).astype(ml_dtypes.float8_e4m3)

    return {
        "iota": iota,
        "tab": tab16,
        "ones": np.ones((1, QCOLS), np.float16),
        "convw": convw.astype(np.float16),
        "tailw": tailw.astype(np.float16),
        "wstack": wstack_t,
        "wg8": wg8,
        "hbT": hbT,
    }


def _prep_ids(char_ids):
    ids = np.asarray(char_ids).reshape(B * S, L)
    ids_pad = np.zeros((B * S, LP), np.int32)
    ids_pad[:, :L] = ids
    nq = W // QW
    per_core = []
    for c in range(NCORES):
        flat = ids_pad[c * W : (c + 1) * W].reshape(-1).astype(np.float16)
        chunks = np.zeros((nq, 128, QCOLS), np.float16)
        for q in range(nq):
            chunks[q, :, :] = flat[q * QCOLS : (q + 1) * QCOLS][None, :]
        per_core.append(chunks)
    return per_core


def _run(inputs, trace=False):
    from concourse.bass_utils import run_bass_kernel_spmd

    if "prog" not in _prog_cache:
        _prog_cache["prog"] = _build_program()
    nc = _prog_cache["prog"]

    shared = _prep_weights(inputs)
    idsb = _prep_ids(inputs["char_ids"])
    in_maps = [dict(shared, idsb=idsb[c]) for c in range(NCORES)]
    br = run_bass_kernel_spmd(nc, in_maps, list(range(NCORES)), trace=trace)
    outs = [br.results[c]["out"] for c in range(NCORES)]  # [2048, 512] each
    full = np.concatenate([o.T for o in outs], axis=0)  # [4096, 2048]
    return full.reshape(B, S, TOTAL_F).astype(np.float32), br


def kernel(**inputs):
    out, _ = _run(inputs, trace=False)
    return out


# revision 27
# speedup vs baseline: 1.0691x; 1.0065x over previous
"""CharacterCNNEmbedding Trainium2 Bass kernel.

Full inputs -> full output [8, 512, 2048]. Data-parallel over 8 NeuronCores
(512 words each). Per core:
  - fp16 one-hot (DVE is_equal, 512-col tiles for 4x mode) + fp16
    gather-matmuls build char embeddings, ACT-copied straight into the
    merged conv moving matrix rows 0:16
  - 6 quarter-level shift DMAs replicate the embedding into rows 16..111
    (tap offsets 1..6); row 112 holds ones (folds conv bias), loaded once
  - one fp16 matmul per (filter-tile, position-chunk), K=113, evaluates all
    7 conv widths at 44 positions; 17 masked tail matmuls cover positions
    44..49 for widths < 7
  - relu+max-pool split between DVE direct reduce and ACT-relu->fp16 staging
    with DVE 2x fold tree
  - 2 highway layers + projection as 2048x2048 fp16 matmuls; weights
    streamed as one [128, 2048] DMA per (matrix, out-tile) (80 DMAs total
    instead of 1280 per-k-tile loads), ACT relu/sigmoid with fused bias,
    DVE gate combine
"""
import sys

sys.path.insert(0, "/opt/trn_rl_repo")
import numpy as np

B, S, L = 8, 512, 50
LP = 56                      # padded word length (shifts stay in-word)
NCORES = 8
W = 512                      # words per core
QW = 128                     # words per quarter
NQ = W // QW
QCOLS = QW * LP              # 7168
COLS = W * LP                # 28672
CD = 16
TOTAL_F = 2048
NMT = TOTAL_F // 128         # 16 filter tiles
VCHUNKS = 3                  # vocab 262 -> 3 chunks of 128
ET = 512                     # emb tile cols (even -> DVE 4x is_equal)
NE = QCOLS // ET             # 14
CHW = 8                      # conv chunk words (352 cols)
NCH = QW // CHW              # 16 chunks per quarter
GRP = 2                      # chunks per PSUM group
FILTERS = [(1, 32), (2, 32), (3, 64), (4, 128), (5, 256), (6, 512), (7, 1024)]
TAIL_TILES = (
    [(44, j) for j in range(8)]
    + [(45, j) for j in range(4)]
    + [(46, j) for j in range(2)]
    + [(47, 0), (48, 0), (49, 0)]
)
DIRECT_M = (5, 11)            # m-tiles on DVE direct-reduce path; rest go
                              # ACT-relu staging + DVE fold tree (interleaved
                              # so both consumer engines run concurrently)

_prog_cache = {}


def _build_program(n_words=W):
    import concourse.tile as tile
    from concourse import bacc, mybir

    F32 = mybir.dt.float32
    F16 = mybir.dt.float16
    F8 = mybir.dt.float8e4
    DR = mybir.MatmulPerfMode.DoubleRow
    nq = n_words // QW

    nc = bacc.Bacc("TRN2", target_bir_lowering=False, debug=False)

    idsb_d = nc.dram_tensor("idsb", [nq, 128, QCOLS], F16, kind="ExternalInput").ap()
    iota_d = nc.dram_tensor("iota", [128, VCHUNKS], F32, kind="ExternalInput").ap()
    tab_d = nc.dram_tensor("tab", [128, VCHUNKS * CD], F16, kind="ExternalInput").ap()
    ones_d = nc.dram_tensor("ones", [1, QCOLS], F16, kind="ExternalInput").ap()
    convw_d = nc.dram_tensor("convw", [113, TOTAL_F], F16, kind="ExternalInput").ap()
    tailw_d = nc.dram_tensor(
        "tailw", [113, len(TAIL_TILES) * 128], F16, kind="ExternalInput"
    ).ap()
    wstack_d = nc.dram_tensor(
        "wstack", [5, NMT, 128, NMT * 128], F16, kind="ExternalInput"
    ).ap()
    wg8_d = nc.dram_tensor(
        "wg8", [2, NMT, 128, NMT * 128], F8, kind="ExternalInput"
    ).ap()
    hbT_d = nc.dram_tensor("hbT", [128, 5 * NMT], F32, kind="ExternalInput").ap()
    out_d = nc.dram_tensor("out", [TOTAL_F, n_words], F32, kind="ExternalOutput").ap()

    AF = mybir.ActivationFunctionType
    OP = mybir.AluOpType
    AX = mybir.AxisListType

    with tile.TileContext(nc) as tc:
        with (
            tc.tile_pool(name="const", bufs=1) as cpool,
            tc.tile_pool(name="h", bufs=1) as hp,
            tc.tile_pool(name="ws", bufs=4) as wsp,
            tc.tile_pool(name="tmp", bufs=2) as tmpp,
            tc.tile_pool(name="outp", bufs=2) as outp,
        ):
            # emb-path consts first so the first quarter starts ASAP
            iota_sb = cpool.tile([128, VCHUNKS], F32)
            nc.sync.dma_start(iota_sb[:], iota_d[:])
            tab_sb = cpool.tile([128, VCHUNKS * CD], F16)
            nc.sync.dma_start(tab_sb[:], tab_d[:])

            # two persistent mov buffers (ones row loaded once each)
            movs = [cpool.tile([128, QCOLS], F16, name=f"mov{i}") for i in range(2)]
            for mv in movs:
                nc.sync.dma_start(mv[112:113, :], ones_d[:])

            convw_sb = cpool.tile([113, TOTAL_F], F16)
            nc.scalar.dma_start(convw_sb[:], convw_d[:])
            tailw_sb = cpool.tile([113, len(TAIL_TILES) * 128], F16)
            nc.scalar.dma_start(tailw_sb[:], tailw_d[:])
            hbT_sb = cpool.tile([128, 5 * NMT], F32)
            nc.scalar.dma_start(hbT_sb[:], hbT_d[:])

            h0 = [hp.tile([128, W], F16, name=f"ha_{k}") for k in range(NMT)]

            # ---------------- conv + embedding phase ----------------
            with (
                tc.tile_pool(name="ids", bufs=2) as idsp,
                tc.tile_pool(name="oh", bufs=9) as ohp,
                tc.tile_pool(name="stg", bufs=2) as stgp,
                tc.tile_pool(name="fold", bufs=3) as foldp,
                tc.tile_pool(name="embp", bufs=2, space="PSUM") as embpp,
                tc.tile_pool(name="convp", bufs=3, space="PSUM") as convpp,
            ):
                for q in range(nq):
                    mov = movs[q % 2]
                    ids_sb = idsp.tile([128, QCOLS], F16, name="ids")
                    for ic in range(4):
                        c_lo = ic * (QCOLS // 4)
                        c_hi = (ic + 1) * (QCOLS // 4)
                        nc.sync.dma_start(
                            ids_sb[:, c_lo:c_hi], idsb_d[q][:, c_lo:c_hi]
                        )

                    for e in range(NE):
                        c0 = ET * e
                        embp = embpp.tile([16, ET], F32, name="embp")
                        for v in range(VCHUNKS):
                            oh = ohp.tile([128, ET], F16, name="oh")
                            nc.vector.tensor_scalar(
                                oh[:],
                                ids_sb[:, c0 : c0 + ET],
                                iota_sb[:, v : v + 1],
                                None,
                                op0=OP.is_equal,
                            )
                            nc.tensor.matmul(
                                embp[:],
                                tab_sb[:, v * CD : (v + 1) * CD],
                                oh[:],
                                start=(v == 0),
                                stop=(v == VCHUNKS - 1),
                            )
                        nc.scalar.activation(
                            mov[0:16, c0 : c0 + ET], embp[:], AF.Copy
                        )

                    # quarter-level shift DMAs: rows 16dt..16dt+16 = emb
                    # shifted left dt chars (taps 1..6; tap 0 is rows 0:16).
                    # Emitted in column halves so conv on words 0..63 can
                    # start after e-tiles 0..7 instead of all 14.
                    H = QCOLS // 2
                    for dt in range(1, 7):
                        nc.sync.dma_start(
                            mov[16 * dt : 16 * dt + 16, 0:H],
                            mov[0:16, dt : H + dt],
                        )
                    for dt in range(1, 7):
                        nc.sync.dma_start(
                            mov[16 * dt : 16 * dt + 16, H : QCOLS - dt],
                            mov[0:16, H + dt : QCOLS],
                        )

                    movv = mov[0:113, :].rearrange("p (w l) -> p w l", l=LP)

                    # conv per filter tile
                    for m in range(NMT):
                        staged = m not in DIRECT_M
                        wsl = convw_sb[:, m * 128 : (m + 1) * 128]
                        if staged:
                            stg = stgp.tile([128, QW, 44], F16, name="stg")
                        else:
                            rm = foldp.tile([128, QW], F32, name="rm")
                        ci = 0
                        while ci < NCH:
                            ng = min(GRP, NCH - ci)
                            cp = convpp.tile([128, GRP, 512], F32, name="cv")
                            for i in range(ng):
                                nc.tensor.matmul(
                                    cp[:, i, 0 : CHW * 44],
                                    wsl,
                                    movv[:, (ci + i) * CHW : (ci + i + 1) * CHW, 0:44],
                                    start=True,
                                    stop=True,
                                )
                            src4 = cp[:, 0:ng, 0 : CHW * 44].rearrange(
                                "p c (w l) -> p c w l", l=44
                            )
                            wlo = ci * CHW
                            whi = (ci + ng) * CHW
                            if staged:
                                nc.scalar.activation(
                                    stg[:, wlo:whi, :].rearrange(
                                        "p (c w) l -> p c w l", c=ng
                                    ),
                                    src4,
                                    AF.Relu,
                                )
                            else:
                                nc.vector.tensor_reduce(
                                    rm[:, wlo:whi].rearrange("p (c w) -> p c w", c=ng),
                                    src4,
                                    op=OP.max,
                                    axis=AX.X,
                                )
                            ci += ng
                        hslice = h0[m][:, q * QW : (q + 1) * QW]
                        if staged:
                            # 44 -> 24 via overlapped max (cols 20..23 counted
                            # twice; harmless for max), then 24 -> 12 -> 6 -> 1
                            t24 = foldp.tile([128, QW, 24], F16, name="t24")
                            nc.vector.tensor_max(
                                t24[:], stg[:, :, 0:24], stg[:, :, 20:44]
                            )
                            f12 = foldp.tile([128, QW, 12], F16, name="fold")
                            nc.vector.tensor_max(
                                f12[:], t24[:, :, 0:12], t24[:, :, 12:24]
                            )
                            f6 = foldp.tile([128, QW, 6], F16, name="fold")
                            nc.vector.tensor_max(f6[:], f12[:, :, 0:6], f12[:, :, 6:12])
                            nc.vector.tensor_reduce(
                                hslice, f6, op=OP.max, axis=AX.X
                            )
                        else:
                            nc.scalar.activation(hslice, rm[:], AF.Relu)

                    # tail positions 44..49 (masked weights)
                    for idx, (tt, j) in enumerate(TAIL_TILES):
                        tp = convpp.tile([128, GRP, 512], F32, name="cv")
                        nc.tensor.matmul(
                            tp[:, 0, 0:QW],
                            tailw_sb[:, idx * 128 : (idx + 1) * 128],
                            movv[:, :, tt],
                            start=True,
                            stop=True,
                        )
                        hs = h0[j][:, q * QW : (q + 1) * QW]
                        nc.vector.tensor_max(hs, hs, tp[:, 0, 0:QW])

            # ---------------- highway + projection ----------------
            with tc.tile_pool(name="hw", bufs=4, space="PSUM") as hwp:
                hin = h0
                h8 = [hp.tile([128, 2, W], F8, name=f"h8_{t}") for t in range(NMT // 2)]
                for layer in range(2):
                    # fp8 pair-interleaved copy of hin for DoubleRow gate chain
                    for k in range(NMT):
                        nc.vector.tensor_copy(h8[k // 2][:, k % 2, :], hin[k][:])
                    hout = [
                        hp.tile([128, W], F16, name=f"h{'b' if layer == 0 else 'a'}_{k}")
                        for k in range(NMT)
                    ]
                    for m in range(NMT):
                        wt = wsp.tile([128, NMT * 128], F16, name="wt")
                        nc.gpsimd.dma_start(wt[:], wstack_d[2 * layer, m])
                        wg = wsp.tile([128, NMT * 128], F8, name="wg")
                        nc.gpsimd.dma_start(wg[:], wg8_d[layer, m])
                        wgv = wg[:].rearrange("p (t s c) -> p t s c", s=2, c=128)
                        pt = hwp.tile([128, 512], F32, name="pt")
                        pg = hwp.tile([128, 512], F32, name="pg")
                        for k in range(NMT):
                            nc.tensor.matmul(
                                pt[:, 0:n_words],
                                wt[:, k * 128 : (k + 1) * 128],
                                hin[k][:],
                                start=(k == 0),
                                stop=(k == NMT - 1),
                            )
                        for t in range(NMT // 2):
                            nc.tensor.matmul(
                                pg[:, 0:n_words],
                                wgv[:, t],
                                h8[t][:],
                                start=(t == 0),
                                stop=(t == NMT // 2 - 1),
                                perf_mode=DR,
                            )
                        t_sb = tmpp.tile([128, W], F16, name="t_sb")
                        nc.scalar.activation(
                            t_sb[:],
                            pt[:, 0:n_words],
                            AF.Relu,
                            bias=hbT_sb[:, 2 * layer * NMT + m : 2 * layer * NMT + m + 1],
                        )
                        g_sb = tmpp.tile([128, W], F16, name="g_sb")
                        nc.scalar.activation(
                            g_sb[:],
                            pg[:, 0:n_words],
                            AF.Sigmoid,
                            bias=hbT_sb[
                                :, (2 * layer + 1) * NMT + m : (2 * layer + 1) * NMT + m + 1
                            ],
                        )
                        d_sb = tmpp.tile([128, W], F16, name="de")
                        nc.vector.tensor_sub(d_sb[:], t_sb[:], hin[m][:])
                        e_sb = tmpp.tile([128, W], F16, name="de")
                        nc.vector.tensor_mul(e_sb[:], g_sb[:], d_sb[:])
                        nc.vector.tensor_add(hout[m][:], hin[m][:], e_sb[:])
                    hin = hout

                for m in range(NMT):
                    wp = wsp.tile([128, NMT * 128], F16, name="wt")
                    nc.gpsimd.dma_start(wp[:], wstack_d[4, m])
                    pp = hwp.tile([128, 512], F32, name="pt")
                    for k in range(NMT):
                        nc.tensor.matmul(
                            pp[:, 0:n_words],
                            wp[:, k * 128 : (k + 1) * 128],
                            hin[k][:],
                            start=(k == 0),
                            stop=(k == NMT - 1),
                        )
                    o_sb = outp.tile([128, W], F32, name="o_sb")
                    nc.vector.tensor_scalar_add(
                        o_sb[:], pp[:, 0:n_words],
                        hbT_sb[:, 4 * NMT + m : 4 * NMT + m + 1],
                    )
                    nc.sync.dma_start(out_d[m * 128 : (m + 1) * 128, :], o_sb[:])

    nc.compile()
    return nc


def _prep_weights(inputs):
    """Host-side weight marshalling (layout + fp16 rounding)."""
    f32 = np.float32
    table = np.asarray(inputs["char_table"], f32).copy()
    table[0] = 0.0
    tab16 = np.zeros((128, VCHUNKS * CD), np.float16)
    for v in range(VCHUNKS):
        rows = table[128 * v : min(128 * (v + 1), table.shape[0])]
        tab16[: rows.shape[0], v * CD : (v + 1) * CD] = rows.astype(np.float16)
    iota = np.zeros((128, VCHUNKS), f32)
    for v in range(VCHUNKS):
        iota[:, v] = np.arange(128) + 128 * v

    convw = np.zeros((113, TOTAL_F), f32)
    offs = np.concatenate([[0], np.cumsum([nf for _, nf in FILTERS])])
    widths = np.repeat([w for w, _ in FILTERS], [nf for _, nf in FILTERS])
    for i, (w, nf) in enumerate(FILTERS):
        cw = np.asarray(inputs[f"conv_w{i}"], f32)  # [nf, 16, w]
        for dt in range(w):
            convw[dt * CD : (dt + 1) * CD, offs[i] : offs[i] + nf] = cw[:, :, dt].T
        convw[112, offs[i] : offs[i] + nf] = np.asarray(inputs[f"conv_b{i}"], f32)
    tailw = np.zeros((113, len(TAIL_TILES) * 128), f32)
    for idx, (tt, j) in enumerate(TAIL_TILES):
        blk = convw[:, 128 * j : 128 * (j + 1)].copy()
        blk[:, widths[128 * j : 128 * (j + 1)] > (50 - tt)] = 0.0
        tailw[:, 128 * idx : 128 * (idx + 1)] = blk

    wstack = np.stack(
        [
            np.asarray(inputs["hw0_tw"], f32).T,
            np.asarray(inputs["hw0_gw"], f32).T,
            np.asarray(inputs["hw1_tw"], f32).T,
            np.asarray(inputs["hw1_gw"], f32).T,
            np.asarray(inputs["proj_w"], f32).T,
        ]
    ).astype(np.float16)
    # pre-tile: [5, 2048, 2048] -> [5, mt, 128(kin rows), kt*128(mout cols)]
    # so one [128, 2048] DMA fetches all 16 k-tiles for out-tile mt
    wstack_t = np.ascontiguousarray(
        wstack.reshape(5, NMT, 128, NMT, 128)
        .transpose(0, 3, 2, 1, 4)
        .reshape(5, NMT, 128, NMT * 128)
    )
    hb = [
        np.asarray(inputs["hw0_tb"], f32),
        np.asarray(inputs["hw0_gb"], f32),
        np.asarray(inputs["hw1_tb"], f32),
        np.asarray(inputs["hw1_gb"], f32),
        np.asarray(inputs["proj_b"], f32),
    ]
    hbT = np.zeros((128, 5 * NMT), f32)
    for p_i in range(5):
        for m in range(NMT):
            hbT[:, p_i * NMT + m] = hb[p_i][m * 128 : (m + 1) * 128]

    import ml_dtypes

    wg8 = np.zeros((2, NMT, 128, NMT * 128), ml_dtypes.float8_e4m3)
    for li, gname in enumerate(["hw0_gw", "hw1_gw"]):
        GT = np.asarray(inputs[gname], f32).T  # [in, out]
        # [t, s, r, m, c] -> [m, r, t, s, c]
        pk = (
            GT.reshape(NMT // 2, 2, 128, NMT, 128)
            .transpose(3, 2, 0, 1, 4)
            .reshape(NMT, 128, NMT * 128)
        )
        wg8[li] = np.clip(pk, -240, 240).astype(ml_dtypes.float8_e4m3)

    return {
        "iota": iota,
        "tab": tab16,
        "ones": np.ones((1, QCOLS), np.float16),
        "convw": convw.astype(np.float16),
        "tailw": tailw.astype(np.float16),
        "wstack": wstack_t,
        "wg8": wg8,
        "hbT": hbT,
    }


def _prep_ids(char_ids):
    ids = np.asarray(char_ids).reshape(B * S, L)
    ids_pad = np.zeros((B * S, LP), np.int32)
    ids_pad[:, :L] = ids
    nq = W // QW
    per_core = []
    for c in range(NCORES):
        flat = ids_pad[c * W : (c + 1) * W].reshape(-1).astype(np.float16)
        chunks = np.zeros((nq, 128, QCOLS), np.float16)
        for q in range(nq):
            chunks[q, :, :] = flat[q * QCOLS : (q + 1) * QCOLS][None, :]
        per_core.append(chunks)
    return per_core


def _run(inputs, trace=False):
    from concourse.bass_utils import run_bass_kernel_spmd

    if "prog" not in _prog_cache:
        _prog_cache["prog"] = _build_program()
    nc = _prog_cache["prog"]

    shared = _prep_weights(inputs)
    idsb = _prep_ids(inputs["char_ids"])
    in_maps = [dict(shared, idsb=idsb[c]) for c in range(NCORES)]
    br = run_bass_kernel_spmd(nc, in_maps, list(range(NCORES)), trace=trace)
    outs = [br.results[c]["out"] for c in range(NCORES)]  # [2048, 512] each
    full = np.concatenate([o.T for o in outs], axis=0)  # [4096, 2048]
    return full.reshape(B, S, TOTAL_F).astype(np.float32), br


def kernel(**inputs):
    out, _ = _run(inputs, trace=False)
    return out
